# revision 1
# baseline (speedup 1.0000x reference)
"""Trainium2 Bass kernel for ContinuousPlanarFlow.

Computes, for z [B, D], W1 [W, D], b1 [W], W2 [D, W]:
    pre     = z @ W1.T + b1            [B, W]
    dz_dt   = softplus(pre) @ W2.T     [B, D]
    dlogpz  = -(sigmoid(pre) @ M)      [B, 1],  M_j = sum_i W1[j,i] W2[i,j]

Key identities used on device (TRN2 has no softplus activation table):
    softplus(pre) = -ln(1 - sigmoid(pre))
so with s = sigmoid(pre) (ACT, sigmoid table), g = ln(1 - s) (ACT, ln table):
    dz_dt = g @ (-W2.T)            (negation folded into host-side weights)
    dlogpz = (-M) @ s              (PE matvec with negated M)

Sharding: data-parallel over batch across 8 cores (8192 rows each); weights
replicated. All activations live on-chip in a transposed layout
(feature j on partitions, batch on the free dim) so no transposes are needed
on device; z is transposed/fp16-cast on the host as part of sharding.

B, D, W = 65536, 256, 1024 (hardcoded).
"""

import numpy as np

B, D, W = 65536, 256, 1024
N_CORES = 8
BC = B // N_CORES          # batch rows per core = 8192
NB = 4096                  # batch columns per super-chunk
N_SC = BC // NB            # super-chunks per core
NJ = W // 128              # 8 j-tiles (feature dim on partitions)
NK = D // 128              # 2 k-tiles of the z contraction

_CACHE = {}


def _build_program():
    import concourse.mybir as mybir
    import concourse.tile as tile
    from concourse import bacc

    fp16 = mybir.dt.float16
    fp32 = mybir.dt.float32
    AF = mybir.ActivationFunctionType

    nc = bacc.Bacc(
        "TRN2",
        target_bir_lowering=False,
        debug=False,
        enable_asserts=True,
        num_devices=N_CORES,
    )

    zT = nc.dram_tensor("zT", [D, BC], fp16, kind="ExternalInput")
    w1t = nc.dram_tensor("w1t", [D, W], fp16, kind="ExternalInput")
    w2tn = nc.dram_tensor("w2tn", [W, D], fp16, kind="ExternalInput")
    b1c = nc.dram_tensor("b1c", [128, NJ], fp32, kind="ExternalInput")
    mneg = nc.dram_tensor("mneg", [128, NJ], fp16, kind="ExternalInput")
    dz = nc.dram_tensor("dz", [BC, D], fp32, kind="ExternalOutput")
    dlp = nc.dram_tensor("dlp", [1, BC], fp32, kind="ExternalOutput")

    with tile.TileContext(nc) as tc:
        with (
            tc.tile_pool(name="const", bufs=1) as cpool,
            tc.tile_pool(name="zt", bufs=4) as ztpool,
            tc.tile_pool(name="s", bufs=9) as spool,
            tc.tile_pool(name="g", bufs=8) as gpool,
            tc.tile_pool(name="dzs", bufs=3) as dzspool,
            tc.tile_pool(name="trs", bufs=3) as trspool,
            tc.tile_pool(name="pre", bufs=2, space="PSUM") as prepool,
            tc.tile_pool(name="dzp", bufs=2, space="PSUM") as dzppool,
            tc.tile_pool(name="trp", bufs=2, space="PSUM") as trppool,
        ):
            # ---- preload replicated weights ----
            w1_sb = []
            for k in range(NK):
                t = cpool.tile([128, W], fp16, name=f"w1_{k}", tag=f"w1_{k}")
                nc.sync.dma_start(t[:], w1t.ap()[k * 128 : (k + 1) * 128, :])
                w1_sb.append(t)
            w2_sb = []
            for j in range(NJ):
                t = cpool.tile([128, D], fp16, name=f"w2_{j}", tag=f"w2_{j}")
                nc.sync.dma_start(t[:], w2tn.ap()[j * 128 : (j + 1) * 128, :])
                w2_sb.append(t)
            b1_sb = cpool.tile([128, NJ], fp32, name="b1_sb", tag="b1_sb")
            nc.sync.dma_start(b1_sb[:], b1c.ap()[:])
            mn_sb = cpool.tile([128, NJ], fp16, name="mn_sb", tag="mn_sb")
            nc.sync.dma_start(mn_sb[:], mneg.ap()[:])

            for sc in range(N_SC):
                c0 = sc * NB
                # ---- stage zT chunk ----
                zt = []
                for k in range(NK):
                    t = ztpool.tile([128, NB], fp16, name=f"zt{sc}_{k}", tag="zt")
                    nc.sync.dma_start(t[:], zT.ap()[k * 128 : (k + 1) * 128, c0 : c0 + NB])
                    zt.append(t)

                # ---- phase 1: pre = W1 @ zT (PSUM), s = sigmoid(pre + b1) ----
                s_tiles = []
                for j in range(NJ):
                    s_t = spool.tile([128, NB], fp16, name=f"s{sc}_{j}", tag="s")
                    s_tiles.append(s_t)
                for j in range(NJ):
                    for nb in range(NB // 1024):
                        pre_t = prepool.tile([128, 1024], fp32, name=f"pre{sc}_{j}_{nb}", tag="pre")
                        for k in range(NK):
                            for p in range(2):
                                nc.tensor.matmul(
                                    pre_t[:, p * 512 : (p + 1) * 512],
                                    w1_sb[k][:, j * 128 : (j + 1) * 128],
                                    zt[k][:, nb * 1024 + p * 512 : nb * 1024 + (p + 1) * 512],
                                    start=(k == 0),
                                    stop=(k == NK - 1),
                                )
                        nc.scalar.activation(
                            s_tiles[j][:, nb * 1024 : (nb + 1) * 1024],
                            pre_t[:],
                            AF.Sigmoid,
                            bias=b1_sb[:, j : j + 1],
                        )

                # ---- phase 1.5: dlogpz = (-M) @ s  (PE matvec, fp32 PSUM acc) ----
                for nb in range(NB // 512):
                    tr_t = trppool.tile([1, 512], fp32, name=f"trp{sc}_{nb}", tag="trp")
                    for j in range(NJ):
                        nc.tensor.matmul(
                            tr_t[:],
                            mn_sb[:, j : j + 1],
                            s_tiles[j][:, nb * 512 : (nb + 1) * 512],
                            start=(j == 0),
                            stop=(j == NJ - 1),
                        )
                    tr_sb = trspool.tile([1, 512], fp32, name=f"trs{sc}_{nb}", tag="trs")
                    nc.vector.tensor_copy(tr_sb[:], tr_t[:])
                    nc.sync.dma_start(dlp.ap()[0:1, c0 + nb * 512 : c0 + (nb + 1) * 512], tr_sb[:])

                # ---- phase 2: g = ln(1 - s)  ( = -softplus(pre) ) ----
                g_tiles = []
                for j in range(NJ):
                    g_t = gpool.tile([128, NB], fp16, name=f"g{sc}_{j}", tag="g")
                    nc.scalar.activation(g_t[:], s_tiles[j][:], AF.Ln, bias=1.0, scale=-1.0)
                    g_tiles.append(g_t)

                # ---- phase 3: dz = g @ (-W2.T)  (batch on out partitions) ----
                for bs in range(NB // 128):
                    dz_t = dzppool.tile([128, D], fp32, name=f"dzp{sc}_{bs}", tag="dzp")
                    for j in range(NJ):
                        nc.tensor.matmul(
                            dz_t[:],
                            g_tiles[j][:, bs * 128 : (bs + 1) * 128],
                            w2_sb[j][:],
                            start=(j == 0),
                            stop=(j == NJ - 1),
                        )
                    dz_sb = dzspool.tile([128, D], fp32, name=f"dzs{sc}_{bs}", tag="dzs")
                    nc.vector.tensor_copy(dz_sb[:], dz_t[:])
                    nc.sync.dma_start(
                        dz.ap()[c0 + bs * 128 : c0 + (bs + 1) * 128, :], dz_sb[:]
                    )

    nc.compile()
    return nc


def _get_program():
    if "nc" not in _CACHE:
        _CACHE["nc"] = _build_program()
    return _CACHE["nc"]


def kernel(t, z, W1, b1, W2):
    from concourse.bass_utils import run_bass_kernel_spmd

    nc = _get_program()

    z = np.asarray(z, dtype=np.float32)
    W1 = np.asarray(W1, dtype=np.float32)
    b1 = np.asarray(b1, dtype=np.float32)
    W2 = np.asarray(W2, dtype=np.float32)

    # host-side weight prep (tiny): M_j = sum_i W1[j,i] W2[i,j]
    M = np.sum(W1.astype(np.float64) * W2.T.astype(np.float64), axis=1)
    mneg = np.ascontiguousarray((-M).reshape(NJ, 128).T).astype(np.float16)
    b1c = np.ascontiguousarray(b1.reshape(NJ, 128).T).astype(np.float32)
    w1t = np.ascontiguousarray(W1.T).astype(np.float16)
    w2tn = np.ascontiguousarray(-W2.T).astype(np.float16)

    in_maps = []
    for c in range(N_CORES):
        zT_c = np.ascontiguousarray(z[c * BC : (c + 1) * BC].T).astype(np.float16)
        in_maps.append(
            {"zT": zT_c, "w1t": w1t, "w2tn": w2tn, "b1c": b1c, "mneg": mneg}
        )

    res = run_bass_kernel_spmd(nc, in_maps, core_ids=list(range(N_CORES)))

    dz = np.concatenate([res.results[c]["dz"] for c in range(N_CORES)], axis=0)
    dlp = np.concatenate([res.results[c]["dlp"][0] for c in range(N_CORES)])[:, None]
    return dz.astype(np.float32), dlp.astype(np.float32)


# revision 4
# speedup vs baseline: 5.8983x; 5.8983x over previous
"""Trainium2 Bass kernel for ContinuousPlanarFlow.

Computes, for z [B, D], W1 [W, D], b1 [W], W2 [D, W]:
    pre     = z @ W1.T + b1            [B, W]
    dz_dt   = softplus(pre) @ W2.T     [B, D]
    dlogpz  = -(sigmoid(pre) @ M)      [B, 1],  M_j = sum_i W1[j,i] W2[i,j]

Key identities used on device (TRN2 has no softplus activation table):
    softplus(pre) = -ln(1 - sigmoid(pre))
so with s = sigmoid(pre) (ACT, sigmoid table), g = ln(1 - s) (ACT, ln table):
    dz_dt = g @ (-W2.T)            (negation folded into host-side weights)
    dlogpz = (-M) @ s              (PE matvec with negated M)

Sharding: data-parallel over batch across 8 cores (8192 rows each); weights
replicated. All activations live on-chip in a transposed layout
(feature j on partitions, batch on the free dim) so no transposes are needed
on device; z is transposed/fp16-cast on the host as part of sharding.

B, D, W = 65536, 256, 1024 (hardcoded).
"""

import numpy as np

B, D, W = 65536, 256, 1024
N_CORES = 8
BC = B // N_CORES          # batch rows per core = 8192
NB = 4096                  # batch columns per super-chunk
N_SC = BC // NB            # super-chunks per core
NJ = W // 128              # 8 j-tiles (feature dim on partitions)
NK = D // 128              # 2 k-tiles of the z contraction

_CACHE = {}


def _build_program(repeats=1):
    import concourse.mybir as mybir
    import concourse.tile as tile
    from concourse import bacc

    fp16 = mybir.dt.float16
    fp32 = mybir.dt.float32
    AF = mybir.ActivationFunctionType

    nc = bacc.Bacc(
        "TRN2",
        target_bir_lowering=False,
        debug=False,
        enable_asserts=True,
        num_devices=N_CORES,
    )

    zT = nc.dram_tensor("zT", [D, BC], fp16, kind="ExternalInput")
    w1t = nc.dram_tensor("w1t", [D, W], fp16, kind="ExternalInput")
    w2tn = nc.dram_tensor("w2tn", [W, D], fp16, kind="ExternalInput")
    b1c = nc.dram_tensor("b1c", [128, NJ], fp32, kind="ExternalInput")
    mneg = nc.dram_tensor("mneg", [128, NJ], fp16, kind="ExternalInput")
    dz = nc.dram_tensor("dz", [BC, D], fp32, kind="ExternalOutput")
    dlp = nc.dram_tensor("dlp", [1, BC], fp32, kind="ExternalOutput")

    with tile.TileContext(nc) as tc:
        with (
            tc.tile_pool(name="const", bufs=1) as cpool,
            tc.tile_pool(name="zt", bufs=4) as ztpool,
            tc.tile_pool(name="s", bufs=9) as spool,
            tc.tile_pool(name="g", bufs=8) as gpool,
            tc.tile_pool(name="dzs", bufs=3) as dzspool,
            tc.tile_pool(name="trs", bufs=3) as trspool,
            tc.tile_pool(name="pre", bufs=2, space="PSUM") as prepool,
            tc.tile_pool(name="dzp", bufs=2, space="PSUM") as dzppool,
            tc.tile_pool(name="trp", bufs=2, space="PSUM") as trppool,
        ):
            # ---- preload replicated weights ----
            w1_sb = []
            for k in range(NK):
                t = cpool.tile([128, W], fp16, name=f"w1_{k}", tag=f"w1_{k}")
                nc.sync.dma_start(t[:], w1t.ap()[k * 128 : (k + 1) * 128, :])
                w1_sb.append(t)
            w2_sb = []
            for j in range(NJ):
                t = cpool.tile([128, D], fp16, name=f"w2_{j}", tag=f"w2_{j}")
                nc.sync.dma_start(t[:], w2tn.ap()[j * 128 : (j + 1) * 128, :])
                w2_sb.append(t)
            b1_sb = cpool.tile([128, NJ], fp32, name="b1_sb", tag="b1_sb")
            nc.sync.dma_start(b1_sb[:], b1c.ap()[:])
            mn_sb = cpool.tile([128, NJ], fp16, name="mn_sb", tag="mn_sb")
            nc.sync.dma_start(mn_sb[:], mneg.ap()[:])

            for sc in range(N_SC * repeats):
                sc = sc % N_SC
                c0 = sc * NB
                # ---- stage zT chunk ----
                zt = []
                for k in range(NK):
                    t = ztpool.tile([128, NB], fp16, name=f"zt{sc}_{k}", tag="zt")
                    nc.sync.dma_start(t[:], zT.ap()[k * 128 : (k + 1) * 128, c0 : c0 + NB])
                    zt.append(t)

                # ---- phase 1: pre = W1 @ zT (PSUM), s = sigmoid(pre + b1) ----
                s_tiles = []
                for j in range(NJ):
                    s_t = spool.tile([128, NB], fp16, name=f"s{sc}_{j}", tag="s")
                    s_tiles.append(s_t)
                for j in range(NJ):
                    for nb in range(NB // 1024):
                        pre_t = prepool.tile([128, 1024], fp32, name=f"pre{sc}_{j}_{nb}", tag="pre")
                        for k in range(NK):
                            for p in range(2):
                                nc.tensor.matmul(
                                    pre_t[:, p * 512 : (p + 1) * 512],
                                    w1_sb[k][:, j * 128 : (j + 1) * 128],
                                    zt[k][:, nb * 1024 + p * 512 : nb * 1024 + (p + 1) * 512],
                                    start=(k == 0),
                                    stop=(k == NK - 1),
                                )
                        nc.scalar.activation(
                            s_tiles[j][:, nb * 1024 : (nb + 1) * 1024],
                            pre_t[:],
                            AF.Sigmoid,
                            bias=b1_sb[:, j : j + 1],
                        )

                # ---- phase 1.5: dlogpz = (-M) @ s  (PE matvec, fp32 PSUM acc) ----
                for nb in range(NB // 512):
                    tr_t = trppool.tile([1, 512], fp32, name=f"trp{sc}_{nb}", tag="trp")
                    for j in range(NJ):
                        nc.tensor.matmul(
                            tr_t[:],
                            mn_sb[:, j : j + 1],
                            s_tiles[j][:, nb * 512 : (nb + 1) * 512],
                            start=(j == 0),
                            stop=(j == NJ - 1),
                        )
                    tr_sb = trspool.tile([1, 512], fp32, name=f"trs{sc}_{nb}", tag="trs")
                    nc.vector.tensor_copy(tr_sb[:], tr_t[:])
                    nc.sync.dma_start(dlp.ap()[0:1, c0 + nb * 512 : c0 + (nb + 1) * 512], tr_sb[:])

                # ---- phase 2: g = ln(1 - s)  ( = -softplus(pre) ) ----
                g_tiles = []
                for j in range(NJ):
                    g_t = gpool.tile([128, NB], fp16, name=f"g{sc}_{j}", tag="g")
                    nc.scalar.activation(g_t[:], s_tiles[j][:], AF.Ln, bias=1.0, scale=-1.0)
                    g_tiles.append(g_t)

                # ---- phase 3: dz = g @ (-W2.T)  (batch on out partitions) ----
                for bs in range(NB // 128):
                    dz_t = dzppool.tile([128, D], fp32, name=f"dzp{sc}_{bs}", tag="dzp")
                    for j in range(NJ):
                        nc.tensor.matmul(
                            dz_t[:],
                            g_tiles[j][:, bs * 128 : (bs + 1) * 128],
                            w2_sb[j][:],
                            start=(j == 0),
                            stop=(j == NJ - 1),
                        )
                    dz_sb = dzspool.tile([128, D], fp32, name=f"dzs{sc}_{bs}", tag="dzs")
                    nc.vector.tensor_copy(dz_sb[:], dz_t[:])
                    nc.sync.dma_start(
                        dz.ap()[c0 + bs * 128 : c0 + (bs + 1) * 128, :], dz_sb[:]
                    )

    nc.compile()
    return nc


def _get_program(repeats=1):
    key = ("nc", repeats)
    if key not in _CACHE:
        _CACHE[key] = _build_program(repeats)
    return _CACHE[key]


def kernel(t, z, W1, b1, W2):
    from concourse.bass_utils import run_bass_kernel_spmd

    nc = _get_program()

    z = np.asarray(z, dtype=np.float32)
    W1 = np.asarray(W1, dtype=np.float32)
    b1 = np.asarray(b1, dtype=np.float32)
    W2 = np.asarray(W2, dtype=np.float32)

    # host-side weight prep (tiny): M_j = sum_i W1[j,i] W2[i,j]
    M = np.sum(W1.astype(np.float64) * W2.T.astype(np.float64), axis=1)
    mneg = np.ascontiguousarray((-M).reshape(NJ, 128).T).astype(np.float16)
    b1c = np.ascontiguousarray(b1.reshape(NJ, 128).T).astype(np.float32)
    w1t = np.ascontiguousarray(W1.T).astype(np.float16)
    w2tn = np.ascontiguousarray(-W2.T).astype(np.float16)

    in_maps = []
    for c in range(N_CORES):
        zT_c = np.ascontiguousarray(z[c * BC : (c + 1) * BC].T).astype(np.float16)
        in_maps.append(
            {"zT": zT_c, "w1t": w1t, "w2tn": w2tn, "b1c": b1c, "mneg": mneg}
        )

    res = run_bass_kernel_spmd(nc, in_maps, core_ids=list(range(N_CORES)))

    dz = np.concatenate([res.results[c]["dz"] for c in range(N_CORES)], axis=0)
    dlp = np.concatenate([res.results[c]["dlp"][0] for c in range(N_CORES)])[:, None]
    return dz.astype(np.float32), dlp.astype(np.float32)


# revision 9
# speedup vs baseline: 26.1696x; 4.4368x over previous
"""Trainium2 Bass kernel for ContinuousPlanarFlow.

Computes, for z [B, D], W1 [W, D], b1 [W], W2 [D, W]:
    pre     = z @ W1.T + b1            [B, W]
    dz_dt   = softplus(pre) @ W2.T     [B, D]
    dlogpz  = -(sigmoid(pre) @ M)      [B, 1],  M_j = sum_i W1[j,i] W2[i,j]

Key identities used on device (TRN2 has no softplus activation table):
    softplus(pre) = -ln(1 - sigmoid(pre))
so with s = sigmoid(pre) (ACT, sigmoid table), g = ln(1 - s) (ACT, ln table):
    dz_dt = g @ (-W2.T)            (negation folded into host-side weights)
    dlogpz = (-M) @ s              (PE matvec with negated M)

Sharding: data-parallel over batch across 8 cores (8192 rows each); weights
replicated. All activations live on-chip in a transposed layout
(feature j on partitions, batch on the free dim) so no transposes are needed
on device; z is transposed/fp16-cast on the host as part of sharding.

B, D, W = 65536, 256, 1024 (hardcoded).
"""

import numpy as np

B, D, W = 65536, 256, 1024
N_CORES = 8
BC = B // N_CORES          # batch rows per core = 8192
NB = 4096                  # batch columns per super-chunk
N_SC = BC // NB            # super-chunks per core
NJ = W // 128              # 8 j-tiles (feature dim on partitions)
NK = D // 128              # 2 k-tiles of the z contraction

_CACHE = {}


def _build_program(repeats=1, loop=None):
    import concourse.mybir as mybir
    import concourse.tile as tile
    from concourse.tile import add_dep_helper
    from concourse import bacc
    from contextlib import ExitStack

    fp16 = mybir.dt.float16
    fp32 = mybir.dt.float32
    AF = mybir.ActivationFunctionType

    nc = bacc.Bacc(
        "TRN2",
        target_bir_lowering=False,
        debug=False,
        enable_asserts=True,
        num_devices=N_CORES,
    )

    zT = nc.dram_tensor("zT", [D, BC], fp16, kind="ExternalInput")
    w1t = nc.dram_tensor("w1t", [D, W], fp16, kind="ExternalInput")
    w2tn = nc.dram_tensor("w2tn", [W, D], fp16, kind="ExternalInput")
    b1c = nc.dram_tensor("b1c", [128, NJ], fp32, kind="ExternalInput")
    mneg = nc.dram_tensor("mneg", [128, NJ], fp16, kind="ExternalInput")
    dz = nc.dram_tensor("dz", [BC, D], fp32, kind="ExternalOutput")
    dlp = nc.dram_tensor("dlp", [1, BC], fp32, kind="ExternalOutput")

    with tile.TileContext(nc) as tc:
        with (
            tc.tile_pool(name="const", bufs=1) as cpool,
            tc.tile_pool(name="zt", bufs=4) as ztpool,
            tc.tile_pool(name="s", bufs=9) as spool,
            tc.tile_pool(name="g", bufs=8) as gpool,
            tc.tile_pool(name="dzs", bufs=3) as dzspool,
            tc.tile_pool(name="trs", bufs=3) as trspool,
            tc.tile_pool(name="pre", bufs=2, space="PSUM") as prepool,
            tc.tile_pool(name="dzp", bufs=2, space="PSUM") as dzppool,
            tc.tile_pool(name="trp", bufs=2, space="PSUM") as trppool,
        ):
            # ---- preload replicated weights ----
            w1_sb = []
            for k in range(NK):
                t = cpool.tile([128, W], fp16, name=f"w1_{k}", tag=f"w1_{k}")
                nc.sync.dma_start(t[:], w1t.ap()[k * 128 : (k + 1) * 128, :])
                w1_sb.append(t)
            w2_sb = []
            for j in range(NJ):
                t = cpool.tile([128, D], fp16, name=f"w2_{j}", tag=f"w2_{j}")
                nc.sync.dma_start(t[:], w2tn.ap()[j * 128 : (j + 1) * 128, :])
                w2_sb.append(t)
            b1_sb = cpool.tile([128, NJ], fp32, name="b1_sb", tag="b1_sb")
            nc.sync.dma_start(b1_sb[:], b1c.ap()[:])
            mn_sb = cpool.tile([128, NJ], fp16, name="mn_sb", tag="mn_sb")
            nc.sync.dma_start(mn_sb[:], mneg.ap()[:])

            # chain ACT instructions in emission order so the scheduler cannot
            # interleave sigmoid-table and ln-table phases (each interleave
            # costs a ~2.7us ACT table reload)
            prev_act = [None]

            def act(*args, **kwargs):
                inst = nc.scalar.activation(*args, **kwargs)
                if prev_act[0] is not None:
                    add_dep_helper(inst.ins, prev_act[0].ins, sync=False,
                                   reason="act table phase ordering")
                prev_act[0] = inst
                return inst

            lctx = ExitStack()
            if loop is not None:
                lctx.enter_context(tc.For_i(0, loop, 1))

            for sc in range(N_SC * repeats):
                sc = sc % N_SC
                c0 = sc * NB
                # ---- stage zT chunk ----
                zt = []
                for k in range(NK):
                    t = ztpool.tile([128, NB], fp16, name=f"zt{sc}_{k}", tag="zt")
                    nc.sync.dma_start(t[:], zT.ap()[k * 128 : (k + 1) * 128, c0 : c0 + NB])
                    zt.append(t)

                # ---- phase 1: pre = W1 @ zT (PSUM), s = sigmoid(pre + b1) ----
                s_tiles = []
                for j in range(NJ):
                    s_t = spool.tile([128, NB], fp16, name=f"s{sc}_{j}", tag="s")
                    s_tiles.append(s_t)
                for j in range(NJ):
                    for nb in range(NB // 1024):
                        pre_t = prepool.tile([128, 1024], fp32, name=f"pre{sc}_{j}_{nb}", tag="pre")
                        for k in range(NK):
                            for p in range(2):
                                nc.tensor.matmul(
                                    pre_t[:, p * 512 : (p + 1) * 512],
                                    w1_sb[k][:, j * 128 : (j + 1) * 128],
                                    zt[k][:, nb * 1024 + p * 512 : nb * 1024 + (p + 1) * 512],
                                    start=(k == 0),
                                    stop=(k == NK - 1),
                                )
                        act(
                            s_tiles[j][:, nb * 1024 : (nb + 1) * 1024],
                            pre_t[:],
                            AF.Sigmoid,
                            bias=b1_sb[:, j : j + 1],
                        )

                # ---- phase 1.5: dlogpz = (-M) @ s  (PE matvec, fp32 PSUM acc) ----
                for nb in range(NB // 512):
                    tr_t = trppool.tile([1, 512], fp32, name=f"trp{sc}_{nb}", tag="trp")
                    for j in range(NJ):
                        nc.tensor.matmul(
                            tr_t[:],
                            mn_sb[:, j : j + 1],
                            s_tiles[j][:, nb * 512 : (nb + 1) * 512],
                            start=(j == 0),
                            stop=(j == NJ - 1),
                        )
                    tr_sb = trspool.tile([1, 512], fp32, name=f"trs{sc}_{nb}", tag="trs")
                    nc.vector.tensor_copy(tr_sb[:], tr_t[:])
                    nc.sync.dma_start(dlp.ap()[0:1, c0 + nb * 512 : c0 + (nb + 1) * 512], tr_sb[:])

                # ---- phase 2: g = ln(1 - s)  ( = -softplus(pre) ) ----
                g_tiles = []
                for j in range(NJ):
                    g_t = gpool.tile([128, NB], fp16, name=f"g{sc}_{j}", tag="g")
                    act(g_t[:], s_tiles[j][:], AF.Ln, bias=1.0, scale=-1.0)
                    g_tiles.append(g_t)

                # ---- phase 3: dz = g @ (-W2.T)  (batch on out partitions) ----
                for bs in range(NB // 128):
                    dz_t = dzppool.tile([128, D], fp32, name=f"dzp{sc}_{bs}", tag="dzp")
                    for j in range(NJ):
                        nc.tensor.matmul(
                            dz_t[:],
                            g_tiles[j][:, bs * 128 : (bs + 1) * 128],
                            w2_sb[j][:],
                            start=(j == 0),
                            stop=(j == NJ - 1),
                        )
                    dz_sb = dzspool.tile([128, D], fp32, name=f"dzs{sc}_{bs}", tag="dzs")
                    nc.vector.tensor_copy(dz_sb[:], dz_t[:])
                    nc.sync.dma_start(
                        dz.ap()[c0 + bs * 128 : c0 + (bs + 1) * 128, :], dz_sb[:]
                    )

            lctx.close()

    nc.compile()
    return nc


def _get_program(repeats=1):
    key = ("nc", repeats)
    if key not in _CACHE:
        _CACHE[key] = _build_program(repeats)
    return _CACHE[key]


def kernel(t, z, W1, b1, W2):
    from concourse.bass_utils import run_bass_kernel_spmd

    nc = _get_program()

    z = np.asarray(z, dtype=np.float32)
    W1 = np.asarray(W1, dtype=np.float32)
    b1 = np.asarray(b1, dtype=np.float32)
    W2 = np.asarray(W2, dtype=np.float32)

    # host-side weight prep (tiny): M_j = sum_i W1[j,i] W2[i,j]
    M = np.sum(W1.astype(np.float64) * W2.T.astype(np.float64), axis=1)
    mneg = np.ascontiguousarray((-M).reshape(NJ, 128).T).astype(np.float16)
    b1c = np.ascontiguousarray(b1.reshape(NJ, 128).T).astype(np.float32)
    w1t = np.ascontiguousarray(W1.T).astype(np.float16)
    w2tn = np.ascontiguousarray(-W2.T).astype(np.float16)

    in_maps = []
    for c in range(N_CORES):
        zT_c = np.ascontiguousarray(z[c * BC : (c + 1) * BC].T).astype(np.float16)
        in_maps.append(
            {"zT": zT_c, "w1t": w1t, "w2tn": w2tn, "b1c": b1c, "mneg": mneg}
        )

    res = run_bass_kernel_spmd(nc, in_maps, core_ids=list(range(N_CORES)))

    dz = np.concatenate([res.results[c]["dz"] for c in range(N_CORES)], axis=0)
    dlp = np.concatenate([res.results[c]["dlp"][0] for c in range(N_CORES)])[:, None]
    return dz.astype(np.float32), dlp.astype(np.float32)


# revision 26
# speedup vs baseline: 27.7011x; 1.0585x over previous
"""Trainium2 Bass kernel for ContinuousPlanarFlow.

Computes, for z [B, D], W1 [W, D], b1 [W], W2 [D, W]:
    pre     = z @ W1.T + b1            [B, W]
    dz_dt   = softplus(pre) @ W2.T     [B, D]
    dlogpz  = -(sigmoid(pre) @ M)      [B, 1],  M_j = sum_i W1[j,i] W2[i,j]

Key identities used on device (TRN2 has no softplus activation table):
    softplus(pre) = -ln(1 - sigmoid(pre))
so with s = sigmoid(pre) (ACT, sigmoid table), g = ln(1 - s) (ACT, ln table):
    dz_dt = g @ (-W2.T)            (negation folded into host-side weights)
    dlogpz = (-M) @ s              (PE matvec with negated M)

Sharding: data-parallel over batch across 8 cores (8192 rows each); weights
replicated. All activations live on-chip in a transposed layout
(feature j on partitions, batch on the free dim) so no transposes are needed
on device; z is transposed/fp16-cast on the host as part of sharding.

B, D, W = 65536, 256, 1024 (hardcoded).
"""

import numpy as np

B, D, W = 65536, 256, 1024
N_CORES = 8
BC = B // N_CORES          # batch rows per core = 8192
NB = 4096                  # batch columns per super-chunk
N_SC = BC // NB            # super-chunks per core
NJ = W // 128              # 8 j-tiles (feature dim on partitions)
NK = D // 128              # 2 k-tiles of the z contraction

_CACHE = {}


def _build_program(repeats=1, loop=None):
    import concourse.mybir as mybir
    import concourse.tile as tile
    from concourse.tile import add_dep_helper
    from concourse import bacc
    from contextlib import ExitStack

    fp16 = mybir.dt.float16
    fp32 = mybir.dt.float32
    AF = mybir.ActivationFunctionType

    nc = bacc.Bacc(
        "TRN2",
        target_bir_lowering=False,
        debug=False,
        enable_asserts=True,
        num_devices=N_CORES,
    )

    zT = nc.dram_tensor("zT", [D, BC], fp16, kind="ExternalInput")
    w1t = nc.dram_tensor("w1t", [D, W], fp16, kind="ExternalInput")
    w2tn = nc.dram_tensor("w2tn", [W, D], fp16, kind="ExternalInput")
    b1c = nc.dram_tensor("b1c", [128, NJ], fp32, kind="ExternalInput")
    mneg = nc.dram_tensor("mneg", [128, NJ], fp16, kind="ExternalInput")
    dz = nc.dram_tensor("dz", [BC, D], fp32, kind="ExternalOutput")
    dlp = nc.dram_tensor("dlp", [1, BC], fp32, kind="ExternalOutput")

    with tile.TileContext(nc) as tc:
        with (
            tc.tile_pool(name="const", bufs=1) as cpool,
            tc.tile_pool(name="zt", bufs=4) as ztpool,
            tc.tile_pool(name="s", bufs=9) as spool,
            tc.tile_pool(name="g", bufs=8) as gpool,
            tc.tile_pool(name="dzs", bufs=3) as dzspool,
            tc.tile_pool(name="trs", bufs=3) as trspool,
            tc.tile_pool(name="pre", bufs=2, space="PSUM") as prepool,
            tc.tile_pool(name="dzp", bufs=2, space="PSUM") as dzppool,
            tc.tile_pool(name="trp", bufs=2, space="PSUM") as trppool,
        ):
            # ---- preload replicated weights ----
            w1_sb = []
            for k in range(NK):
                t = cpool.tile([128, W], fp16, name=f"w1_{k}", tag=f"w1_{k}")
                nc.sync.dma_start(t[:], w1t.ap()[k * 128 : (k + 1) * 128, :])
                w1_sb.append(t)
            # w2/b1/M aren't needed until later: load via the idle ACT queue
            # so the first zT chunk isn't stuck behind them on the sync queue
            w2_sb = []
            for j in range(NJ):
                t = cpool.tile([128, D], fp16, name=f"w2_{j}", tag=f"w2_{j}")
                nc.scalar.dma_start(t[:], w2tn.ap()[j * 128 : (j + 1) * 128, :])
                w2_sb.append(t)
            b1_sb = cpool.tile([128, NJ], fp32, name="b1_sb", tag="b1_sb")
            nc.scalar.dma_start(b1_sb[:], b1c.ap()[:])
            mn_sb = cpool.tile([128, NJ], fp16, name="mn_sb", tag="mn_sb")
            nc.scalar.dma_start(mn_sb[:], mneg.ap()[:])

            # chain ACT instructions in emission order so the scheduler cannot
            # interleave sigmoid-table and ln-table phases (each interleave
            # costs a ~2.7us ACT table reload)
            prev_act = [None]

            def act(*args, **kwargs):
                inst = nc.scalar.activation(*args, **kwargs)
                if prev_act[0] is not None:
                    add_dep_helper(inst.ins, prev_act[0].ins, sync=False,
                                   reason="act table phase ordering")
                prev_act[0] = inst
                return inst

            lctx = ExitStack()
            if loop is not None:
                lctx.enter_context(tc.For_i(0, loop, 1))

            def emit_mm2_group(gt, gc0, bs):
                """One dz accumulation group: 128 batch rows x 256 out dims."""
                dz_t = dzppool.tile([128, D], fp32, name=f"dzp_{gc0}_{bs}", tag="dzp")
                for j in range(NJ):
                    nc.tensor.matmul(
                        dz_t[:],
                        gt[j][:, bs * 128 : (bs + 1) * 128],
                        w2_sb[j][:],
                        start=(j == 0),
                        stop=(j == NJ - 1),
                    )
                dz_sb = dzspool.tile([128, D], fp32, name=f"dzs_{gc0}_{bs}", tag="dzs")
                nc.vector.tensor_copy(dz_sb[:], dz_t[:])
                nc.gpsimd.dma_start(
                    dz.ap()[gc0 + bs * 128 : gc0 + (bs + 1) * 128, :], dz_sb[:]
                )

            NNB = NB // 1024          # 1024-wide nb blocks per super-chunk
            NGRP = NB // 128          # mm2 groups per super-chunk (32)
            LNC = 2048                # ln chunk width (columns per ln sweep)
            NLC = NB // LNC           # ln sweeps per super-chunk

            for it in range(N_SC * repeats):
                sc = it % N_SC
                c0 = sc * NB
                # ---- stage zT chunk (split DMAs so mm1 starts early) ----
                zt = []
                for k in range(NK):
                    t = ztpool.tile([128, NB], fp16, name=f"zt{it}_{k}", tag="zt")
                    for nb in range(NNB):
                        nc.sync.dma_start(
                            t[:, nb * 1024 : (nb + 1) * 1024],
                            zT.ap()[k * 128 : (k + 1) * 128, c0 + nb * 1024 : c0 + (nb + 1) * 1024],
                        )
                    zt.append(t)

                # ---- phase 1 (mm1 + sigmoid), interleaved with prev mm2 ----
                s_tiles = [
                    spool.tile([128, NB], fp16, name=f"s{it}_{j}", tag="s")
                    for j in range(NJ)
                ]
                for nb in range(NNB):
                    for j in range(NJ):
                        pre_t = prepool.tile([128, 1024], fp32, name=f"pre{it}_{j}_{nb}", tag="pre")
                        for k in range(NK):
                            for p in range(2):
                                nc.tensor.matmul(
                                    pre_t[:, p * 512 : (p + 1) * 512],
                                    w1_sb[k][:, j * 128 : (j + 1) * 128],
                                    zt[k][:, nb * 1024 + p * 512 : nb * 1024 + (p + 1) * 512],
                                    start=(k == 0),
                                    stop=(k == NK - 1),
                                )
                        act(
                            s_tiles[j][:, nb * 1024 : (nb + 1) * 1024],
                            pre_t[:],
                            AF.Sigmoid,
                            bias=b1_sb[:, j : j + 1],
                        )
                # ---- dlogpz = (-M) @ s  (PE matvec; fills PE stalls) ----
                for nb in range(NB // 512):
                    tr_t = trppool.tile([1, 512], fp32, name=f"trp{it}_{nb}", tag="trp")
                    for j in range(NJ):
                        nc.tensor.matmul(
                            tr_t[:],
                            mn_sb[:, j : j + 1],
                            s_tiles[j][:, nb * 512 : (nb + 1) * 512],
                            start=(j == 0),
                            stop=(j == NJ - 1),
                        )
                    tr_sb = trspool.tile([1, 512], fp32, name=f"trs{it}_{nb}", tag="trs")
                    nc.vector.tensor_copy(tr_sb[:], tr_t[:])
                    nc.gpsimd.dma_start(dlp.ap()[0:1, c0 + nb * 512 : c0 + (nb + 1) * 512], tr_sb[:])

                # ---- g = ln(1 - s) ( = -softplus(pre) ), chunked in LNC-wide
                # sweeps; each sweep unlocks its mm2 groups so PE overlaps
                # the ln pass chunk-by-chunk (and the kernel tail shrinks) ----
                g_tiles = [
                    gpool.tile([128, NB], fp16, name=f"g{it}_{j}", tag="g")
                    for j in range(NJ)
                ]
                # last super-chunk: halve the final sweep so the kernel tail
                # (mm2 after the last ln) is as short as possible
                if sc == N_SC - 1:
                    sweeps = [(0, LNC), (LNC, LNC // 2), (LNC + LNC // 2, LNC // 2)]
                else:
                    sweeps = [(lc * LNC, LNC) for lc in range(NLC)]
                for off, width in sweeps:
                    sl = slice(off, off + width)
                    for j in range(NJ):
                        act(g_tiles[j][:, sl], s_tiles[j][:, sl], AF.Ln,
                            bias=1.0, scale=-1.0)
                    for bs in range(off // 128, (off + width) // 128):
                        emit_mm2_group(g_tiles, c0, bs)

            lctx.close()

    nc.compile()
    return nc


def _get_program(repeats=1):
    key = ("nc", repeats)
    if key not in _CACHE:
        _CACHE[key] = _build_program(repeats)
    return _CACHE[key]


def kernel(t, z, W1, b1, W2):
    from concourse.bass_utils import run_bass_kernel_spmd

    nc = _get_program()

    z = np.asarray(z, dtype=np.float32)
    W1 = np.asarray(W1, dtype=np.float32)
    b1 = np.asarray(b1, dtype=np.float32)
    W2 = np.asarray(W2, dtype=np.float32)

    # host-side weight prep (tiny): M_j = sum_i W1[j,i] W2[i,j]
    M = np.sum(W1.astype(np.float64) * W2.T.astype(np.float64), axis=1)
    mneg = np.ascontiguousarray((-M).reshape(NJ, 128).T).astype(np.float16)
    b1c = np.ascontiguousarray(b1.reshape(NJ, 128).T).astype(np.float32)
    w1t = np.ascontiguousarray(W1.T).astype(np.float16)
    w2tn = np.ascontiguousarray(-W2.T).astype(np.float16)

    in_maps = []
    for c in range(N_CORES):
        zT_c = np.ascontiguousarray(z[c * BC : (c + 1) * BC].T).astype(np.float16)
        in_maps.append(
            {"zT": zT_c, "w1t": w1t, "w2tn": w2tn, "b1c": b1c, "mneg": mneg}
        )

    res = run_bass_kernel_spmd(nc, in_maps, core_ids=list(range(N_CORES)))

    dz = np.concatenate([res.results[c]["dz"] for c in range(N_CORES)], axis=0)
    dlp = np.concatenate([res.results[c]["dlp"][0] for c in range(N_CORES)])[:, None]
    return dz.astype(np.float32), dlp.astype(np.float32)


# revision 39
# speedup vs baseline: 57.4395x; 2.0735x over previous
"""Trainium2 Bass kernel for ContinuousPlanarFlow.

Computes, for z [B, D], W1 [W, D], b1 [W], W2 [D, W]:
    pre     = z @ W1.T + b1            [B, W]
    dz_dt   = softplus(pre) @ W2.T     [B, D]
    dlogpz  = -(sigmoid(pre) @ M)      [B, 1],  M_j = sum_i W1[j,i] W2[i,j]

Key identities used on device (TRN2 has no softplus activation table):
    softplus(pre) = -ln(1 - sigmoid(pre))
so with s = sigmoid(pre) (ACT, sigmoid table), g = ln(1 - s) (ACT, ln table):
    dz_dt = g @ (-W2.T)            (negation folded into host-side weights)
    dlogpz = (-M) @ s              (PE matvec with negated M)

Sharding: data-parallel over batch across 8 cores (8192 rows each); weights
replicated. All activations live on-chip in a transposed layout
(feature j on partitions, batch on the free dim) so no transposes are needed
on device; z is transposed/fp16-cast on the host as part of sharding.

B, D, W = 65536, 256, 1024 (hardcoded).
"""

import numpy as np

B, D, W = 65536, 256, 1024
N_CORES = 8
BC = B // N_CORES          # batch rows per core = 8192
NB = 4096                  # batch columns per super-chunk
N_SC = BC // NB            # super-chunks per core
NJ = W // 128              # 8 j-tiles (feature dim on partitions)
NK = D // 128              # 2 k-tiles of the z contraction

_CACHE = {}


def _build_program(repeats=1, loop=None):
    import concourse.mybir as mybir
    import concourse.tile as tile
    from concourse.tile import add_dep_helper
    from concourse import bacc
    from contextlib import ExitStack

    fp16 = mybir.dt.float16
    fp32 = mybir.dt.float32
    AF = mybir.ActivationFunctionType

    nc = bacc.Bacc(
        "TRN2",
        target_bir_lowering=False,
        debug=False,
        enable_asserts=True,
        num_devices=N_CORES,
    )

    zT = nc.dram_tensor("zT", [D, BC], fp16, kind="ExternalInput")
    w1t = nc.dram_tensor("w1t", [D, W], fp16, kind="ExternalInput")
    w2tn = nc.dram_tensor("w2tn", [W, D], fp16, kind="ExternalInput")
    b1c = nc.dram_tensor("b1c", [128, NJ], fp32, kind="ExternalInput")
    mneg = nc.dram_tensor("mneg", [128, NJ], fp16, kind="ExternalInput")
    dz = nc.dram_tensor("dz", [BC, D], fp32, kind="ExternalOutput")
    dlp = nc.dram_tensor("dlp", [1, BC], fp32, kind="ExternalOutput")

    with tile.TileContext(nc) as tc:
        with (
            tc.tile_pool(name="const", bufs=1) as cpool,
            tc.tile_pool(name="zt", bufs=4) as ztpool,
            tc.tile_pool(name="s", bufs=9) as spool,
            tc.tile_pool(name="g", bufs=8) as gpool,
            tc.tile_pool(name="dzs", bufs=3) as dzspool,
            tc.tile_pool(name="trs", bufs=3) as trspool,
            tc.tile_pool(name="pre", bufs=2, space="PSUM") as prepool,
            tc.tile_pool(name="dzp", bufs=2, space="PSUM") as dzppool,
            tc.tile_pool(name="trp", bufs=2, space="PSUM") as trppool,
        ):
            # ---- preload replicated weights ----
            w1_sb = []
            for k in range(NK):
                t = cpool.tile([128, W], fp16, name=f"w1_{k}", tag=f"w1_{k}")
                nc.sync.dma_start(t[:], w1t.ap()[k * 128 : (k + 1) * 128, :])
                w1_sb.append(t)
            # w2/b1/M aren't needed until later: load via the idle ACT queue
            # so the first zT chunk isn't stuck behind them on the sync queue
            w2_sb = []
            for j in range(NJ):
                t = cpool.tile([128, D], fp16, name=f"w2_{j}", tag=f"w2_{j}")
                nc.gpsimd.dma_start(t[:], w2tn.ap()[j * 128 : (j + 1) * 128, :])
                w2_sb.append(t)
            b1_sb = cpool.tile([128, NJ], fp32, name="b1_sb", tag="b1_sb")
            nc.scalar.dma_start(b1_sb[:], b1c.ap()[:])
            mn_sb = cpool.tile([128, NJ], fp16, name="mn_sb", tag="mn_sb")
            nc.scalar.dma_start(mn_sb[:], mneg.ap()[:])

            # chain ACT instructions in emission order so the scheduler cannot
            # interleave sigmoid-table and ln-table phases (each interleave
            # costs a ~2.7us ACT table reload)
            prev_act = [None]

            def act(*args, **kwargs):
                inst = nc.scalar.activation(*args, **kwargs)
                if prev_act[0] is not None:
                    add_dep_helper(inst.ins, prev_act[0].ins, sync=False,
                                   reason="act table phase ordering")
                prev_act[0] = inst
                return inst

            warm_in = cpool.tile([128, 1], fp32, name="warm_in", tag="warm_in")
            nc.vector.memset(warm_in[:], 0.0)
            warm_sb = cpool.tile([128, 1], fp16, name="warm_sb", tag="warm_sb")
            act(warm_sb[:], warm_in[:], AF.Sigmoid)

            lctx = ExitStack()
            if loop is not None:
                lctx.enter_context(tc.For_i(0, loop, 1))

            def emit_mm2_group(gt, gc0, bs):
                """One dz accumulation group: 128 batch rows x 256 out dims."""
                dz_t = dzppool.tile([128, D], fp32, name=f"dzp_{gc0}_{bs}", tag="dzp")
                for j in range(NJ):
                    nc.tensor.matmul(
                        dz_t[:],
                        gt[j][:, bs * 128 : (bs + 1) * 128],
                        w2_sb[j][:],
                        start=(j == 0),
                        stop=(j == NJ - 1),
                    )
                dz_sb = dzspool.tile([128, D], fp32, name=f"dzs_{gc0}_{bs}", tag="dzs")
                nc.vector.tensor_copy(dz_sb[:], dz_t[:])
                nc.sync.dma_start(
                    dz.ap()[gc0 + bs * 128 : gc0 + (bs + 1) * 128, :], dz_sb[:]
                )

            NNB = NB // 1024          # 1024-wide nb blocks per super-chunk
            NGRP = NB // 128          # mm2 groups per super-chunk (32)
            LNC = 2048                # ln chunk width (columns per ln sweep)
            NLC = NB // LNC           # ln sweeps per super-chunk
            pending = []              # deferred mm2 groups (g_tiles, c0, bs)

            for it in range(N_SC * repeats):
                sc = it % N_SC
                c0 = sc * NB
                # ---- stage zT chunk (split DMAs so mm1 starts early) ----
                zt = []
                for k in range(NK):
                    t = ztpool.tile([128, NB], fp16, name=f"zt{it}_{k}", tag="zt")
                    for nb in range(NNB):
                        nc.sync.dma_start(
                            t[:, nb * 1024 : (nb + 1) * 1024],
                            zT.ap()[k * 128 : (k + 1) * 128, c0 + nb * 1024 : c0 + (nb + 1) * 1024],
                        )
                    zt.append(t)

                # ---- phase 1 (mm1 + sigmoid), interleaved with prev mm2 ----
                s_tiles = [
                    spool.tile([128, NB], fp16, name=f"s{it}_{j}", tag="s")
                    for j in range(NJ)
                ]
                for nb in range(NNB):
                    for j in range(NJ):
                        pre_t = prepool.tile([128, 1024], fp32, name=f"pre{it}_{j}_{nb}", tag="pre")
                        for k in range(NK):
                            for p in range(2):
                                nc.tensor.matmul(
                                    pre_t[:, p * 512 : (p + 1) * 512],
                                    w1_sb[k][:, j * 128 : (j + 1) * 128],
                                    zt[k][:, nb * 1024 + p * 512 : nb * 1024 + (p + 1) * 512],
                                    start=(k == 0),
                                    stop=(k == NK - 1),
                                )
                        act(
                            s_tiles[j][:, nb * 1024 : (nb + 1) * 1024],
                            pre_t[:],
                            AF.Sigmoid,
                            bias=b1_sb[:, j : j + 1],
                        )
                        # drain leftover mm2 work of the previous super-chunk
                        # into the sigmoid-paced mm1 stalls (after the first
                        # two units so sigmoid un-gates immediately)
                        if j in (1, 3, 5) and pending:
                            emit_mm2_group(*pending.pop(0))
                # ---- dlogpz = (-M) @ s  (PE matvecs; fill sig-era PE slack) ----
                for nb in range(NB // 512):
                    tr_t = trppool.tile([1, 512], fp32, name=f"trp{it}_{nb}", tag="trp")
                    for j in range(NJ):
                        nc.tensor.matmul(
                            tr_t[:],
                            mn_sb[:, j : j + 1],
                            s_tiles[j][:, nb * 512 : (nb + 1) * 512],
                            start=(j == 0),
                            stop=(j == NJ - 1),
                        )
                    tr_sb = trspool.tile([1, 512], fp32, name=f"trs{it}_{nb}", tag="trs")
                    nc.vector.tensor_copy(tr_sb[:], tr_t[:])
                    nc.gpsimd.dma_start(dlp.ap()[0:1, c0 + nb * 512 : c0 + (nb + 1) * 512], tr_sb[:])

                # ---- g = ln(1 - s) ( = -softplus(pre) ), chunked in LNC-wide
                # sweeps; each sweep unlocks its mm2 groups so PE overlaps
                # the ln pass chunk-by-chunk (and the kernel tail shrinks) ----
                g_tiles = [
                    gpool.tile([128, NB], fp16, name=f"g{it}_{j}", tag="g")
                    for j in range(NJ)
                ]
                # last super-chunk: halve the final sweep so the kernel tail
                # (mm2 after the last ln) is as short as possible
                if sc == N_SC - 1:
                    sweeps = [(0, LNC), (LNC, 1024), (LNC + 1024, 512), (LNC + 1536, 512)]
                else:
                    sweeps = [(lc * LNC, LNC) for lc in range(NLC)]
                last_sc = sc == N_SC - 1
                for off, width in sweeps:
                    sl = slice(off, off + width)
                    for j in range(NJ):
                        act(g_tiles[j][:, sl], s_tiles[j][:, sl], AF.Ln,
                            bias=1.0, scale=-1.0)
                    groups = list(range(off // 128, (off + width) // 128))
                    if not last_sc:
                        # defer ~6 groups per sweep into the next phase1
                        keep = max(len(groups) - 6, 0)
                        for bs in groups[keep:]:
                            pending.append((g_tiles, c0, bs))
                        groups = groups[:keep]
                    for bs in groups:
                        emit_mm2_group(g_tiles, c0, bs)

            lctx.close()

    nc.compile()
    return nc


def _get_program(repeats=1):
    key = ("nc", repeats)
    if key not in _CACHE:
        _CACHE[key] = _build_program(repeats)
    return _CACHE[key]


def kernel(t, z, W1, b1, W2):
    from concourse.bass_utils import run_bass_kernel_spmd

    nc = _get_program()

    z = np.asarray(z, dtype=np.float32)
    W1 = np.asarray(W1, dtype=np.float32)
    b1 = np.asarray(b1, dtype=np.float32)
    W2 = np.asarray(W2, dtype=np.float32)

    # host-side weight prep (tiny): M_j = sum_i W1[j,i] W2[i,j]
    M = np.sum(W1.astype(np.float64) * W2.T.astype(np.float64), axis=1)
    mneg = np.ascontiguousarray((-M).reshape(NJ, 128).T).astype(np.float16)
    b1c = np.ascontiguousarray(b1.reshape(NJ, 128).T).astype(np.float32)
    w1t = np.ascontiguousarray(W1.T).astype(np.float16)
    w2tn = np.ascontiguousarray(-W2.T).astype(np.float16)

    in_maps = []
    for c in range(N_CORES):
        zT_c = np.ascontiguousarray(z[c * BC : (c + 1) * BC].T).astype(np.float16)
        in_maps.append(
            {"zT": zT_c, "w1t": w1t, "w2tn": w2tn, "b1c": b1c, "mneg": mneg}
        )

    res = run_bass_kernel_spmd(nc, in_maps, core_ids=list(range(N_CORES)))

    dz = np.concatenate([res.results[c]["dz"] for c in range(N_CORES)], axis=0)
    dlp = np.concatenate([res.results[c]["dlp"][0] for c in range(N_CORES)])[:, None]
    return dz.astype(np.float32), dlp.astype(np.float32)
